# revision 1
# baseline (speedup 1.0000x reference)
"""Trainium2 Bass kernel for nn_DenTargetTransformerConv (GNN message passing).

Strategy (graph/data parallel, dst-owner sharding across 8 NeuronCores):
  - Nodes are partitioned by dst-id range; each core owns N/8 nodes and all
    edges whose dst falls in its range. Cores are fully independent (the
    "halo exchange" of src features is materialized host-side as per-section
    compacted gather tables; the device gathers per-edge rows from them).
  - Per core, own nodes are sorted by in-degree and packed into groups of
    128 (SBUF partition dim). Every node in group g gets K[g] edge slots
    (K[g] = max degree in that group position across all cores, so the 8
    cores share one compiled program). Per-edge q||v rows (512B) are
    fetched with bulk dma_gather instructions; scores, edge softmax
    (max-subtraction elided: scores are O(+-5) in f32), and the weighted
    aggregation run on DVE/ACT with free-axis strided reduces. The small
    per-node linears run on PE with the bias folded in via a ones-row.
"""

import numpy as np

import concourse.bacc as bacc
import concourse.bass as bass
import concourse.tile as tile
from concourse import mybir
from concourse.bass_utils import run_bass_kernel_spmd

F32 = mybir.dt.float32
I16 = mybir.dt.int16
AX = mybir.AxisListType
ALU = mybir.AluOpType
ACTF = mybir.ActivationFunctionType

P = 128
NCORES = 8
HD = 64          # H * D
H, D = 4, 16
IN_F = 64

RUNC = 48        # max slot-columns per merged compute run


# ----------------------------------------------------------------- host prep

def _plan(q_src, v_src, feat, src, dst, ncores):
    n = feat.shape[0]
    npc = n // ncores
    ngrp = (npc + P - 1) // P
    grid = ngrp * P
    ndum = grid - npc

    qv = np.concatenate(
        [np.asarray(q_src, np.float32).reshape(n, HD),
         np.asarray(v_src, np.float32).reshape(n, HD)], axis=1)  # [n, 128]

    src = np.asarray(src).astype(np.int64)
    dst = np.asarray(dst).astype(np.int64)
    order = np.argsort(dst, kind="stable")
    dst_s, src_s = dst[order], src[order]
    bounds = np.searchsorted(dst_s, np.arange(ncores + 1) * npc)

    cores = []
    gmax = np.zeros((ncores, ngrp), np.int64)
    for c in range(ncores):
        lo, hi = bounds[c], bounds[c + 1]
        dstL = dst_s[lo:hi] - c * npc          # ascending
        srcL = src_s[lo:hi]
        deg = np.bincount(dstL, minlength=npc)
        starts = np.concatenate([[0], np.cumsum(deg)])
        rank = np.arange(len(dstL)) - starts[dstL]
        perm = np.argsort(deg, kind="stable")  # ascending degree
        pos_of = np.empty(npc, np.int64)
        pos_of[perm] = ndum + np.arange(npc)
        gd = np.zeros(grid, np.int64)
        gd[ndum:] = deg[perm]
        gmax[c] = gd.reshape(ngrp, P).max(1)
        cores.append(dict(dstL=dstL, srcL=srcL, rank=rank, perm=perm,
                          pos_of=pos_of))

    K = np.maximum(gmax.max(0), 1)             # shared per-group slot count
    colbase = np.concatenate([[0], np.cumsum(K)]).astype(np.int64)
    totc = int(colbase[-1])

    # Per-core halo-exchange buffer: each node's K[g] neighbor qv rows are
    # staged contiguously (rows shared by several owned nodes are replicated
    # per consumer), so the device gather needs one descriptor per node.
    # Row layout: colbase[g]*128 + p*K[g] + k  for slot (group g, node p, k).
    per_core = []
    for c in range(ncores):
        cd = cores[c]
        pos_e = cd["pos_of"][cd["dstL"]]       # grid position of each edge
        g_e = pos_e // P
        p_e = pos_e % P
        col_e = colbase[g_e] + cd["rank"]
        tab = np.zeros((totc * P, 2 * HD), np.float32)
        rows = colbase[g_e] * P + p_e * K[g_e] + cd["rank"]
        tab[rows] = qv[cd["srcL"]]
        mask_flat = np.zeros(totc * P, np.float32)
        mask_flat[col_e * P + p_e] = 1.0
        mask_w = mask_flat.reshape(totc, P).T.copy()               # [128, totc]
        per_core.append(dict(tab=tab, mask=mask_w))

    # featT with ones row, per core, grid-permuted: [IN_F+1, grid]
    featTs = []
    feat = np.asarray(feat, np.float32)
    for c in range(ncores):
        ft = np.zeros((IN_F + 1, grid), np.float32)
        ft[IN_F, :] = 1.0
        perm = cores[c]["perm"]
        ft[:IN_F, ndum:] = feat[c * npc + perm].T
        featTs.append(ft)

    # Merge consecutive equal-K groups into runs of <= RUNC slot-columns;
    # all run APs stay within the 3-free-dim ISA limit via (H,D)->HD and
    # (R,K)->RK collapses.
    runs = []
    g = 0
    while g < ngrp:
        k = int(K[g])
        ge = g + 1
        while ge < ngrp and int(K[ge]) == k and (ge - g + 1) * k <= RUNC:
            ge += 1
        runs.append((g, ge, k))
        g = ge
    mrc = max((g1 - g0) * k for (g0, g1, k) in runs)
    rmax = max(g1 - g0 for (g0, g1, k) in runs)

    # identity gather indices for the largest run, wrapped + replicated
    idx_flat = np.arange(P * rmax, dtype=np.int16)
    idx_w = np.tile(idx_flat.reshape(P * rmax // 16, 16).T, (8, 1))

    return dict(n=n, npc=npc, ngrp=ngrp, grid=grid, ndum=ndum, K=K,
                colbase=colbase, totc=totc, runs=runs, mrc=mrc, rmax=rmax,
                idx_w=idx_w, cores=cores, per_core=per_core, featTs=featTs)


# ------------------------------------------------------------- device build

def _build_nc(plan, ncores):
    ngrp, totc, runs = plan["ngrp"], plan["totc"], plan["runs"]
    grid = plan["grid"]
    mrc = plan["mrc"]
    colbase = plan["colbase"]

    nc = bacc.Bacc("TRN2", target_bir_lowering=False, debug=False,
                   num_devices=ncores)

    featT_d = nc.dram_tensor("featT", [IN_F + 1, grid], F32,
                             kind="ExternalInput").ap()
    rmax = plan["rmax"]
    idx_d = nc.dram_tensor("idx", [P, 8 * rmax], I16,
                           kind="ExternalInput").ap()
    mask_d = nc.dram_tensor("mask", [P, totc], F32, kind="ExternalInput").ap()
    tab_d = nc.dram_tensor("tab", [totc * P, 2 * HD], F32,
                           kind="ExternalInput").ap()
    wk_d = nc.dram_tensor("wk", [IN_F + 1, HD], F32, kind="ExternalInput").ap()
    wsk_d = nc.dram_tensor("wsk", [IN_F + 1, HD], F32, kind="ExternalInput").ap()
    # gate weights / ln params / prelu packed on one row:
    # [wg1' (64) | wg2' (64) | bgate (1) | gamma (64) | beta (64) | prelu_a (1)]
    par_d = nc.dram_tensor("par", [1, 258], F32, kind="ExternalInput").ap()
    out_d = nc.dram_tensor("out", [P, ngrp * HD], F32, kind="ExternalOutput").ap()

    with tile.TileContext(nc) as tc:
        with (
            tc.tile_pool(name="singles", bufs=1) as singles,
            tc.tile_pool(name="psum", bufs=4, space="PSUM") as psum,
            tc.tile_pool(name="qvp", bufs=3) as qvp,
            tc.tile_pool(name="scr", bufs=4) as scr,
            tc.tile_pool(name="exs", bufs=4) as exs,
        ):
            # ---- static loads
            featT = singles.tile([IN_F + 1, grid], F32)
            nc.sync.dma_start(out=featT[:], in_=featT_d[:])
            idx_sb = singles.tile([P, 8 * rmax], I16)
            nc.sync.dma_start(out=idx_sb[:], in_=idx_d[:])
            mask_sb = singles.tile([P, totc], F32)
            nc.sync.dma_start(out=mask_sb[:], in_=mask_d[:])
            wk_sb = singles.tile([IN_F + 1, HD], F32)
            nc.sync.dma_start(out=wk_sb[:], in_=wk_d[:])
            wsk_sb = singles.tile([IN_F + 1, HD], F32)
            nc.sync.dma_start(out=wsk_sb[:], in_=wsk_d[:])
            # broadcast params to all partitions (replicating DMA)
            parb = singles.tile([P, 258], F32)
            nc.gpsimd.dma_start(
                out=parb[:],
                in_=bass.AP(tensor=par_d.tensor, offset=par_d.offset,
                            ap=[[0, P], [1, 258]]))
            wg1 = parb[:, 0:64]
            wg2 = parb[:, 64:128]
            bg = parb[:, 128:129]
            gamma = parb[:, 129:193]
            beta = parb[:, 193:257]
            pa = parb[:, 257:258]

            # ---- per-node linears on PE: k = feat@Wk + bk, skip = feat@Wskip + bskip
            k_sb = singles.tile([P, ngrp * HD], F32)
            skip_sb = singles.tile([P, ngrp * HD], F32)
            for g in range(ngrp):
                lhsT = featT[:, g * P:(g + 1) * P]
                pk = psum.tile([P, HD], F32, tag="pk")
                nc.tensor.matmul(out=pk[:], lhsT=lhsT, rhs=wk_sb[:],
                                 start=True, stop=True)
                nc.scalar.activation(out=k_sb[:, g * HD:(g + 1) * HD],
                                     in_=pk[:], func=ACTF.Copy)
                ps = psum.tile([P, HD], F32, tag="pk")
                nc.tensor.matmul(out=ps[:], lhsT=lhsT, rhs=wsk_sb[:],
                                 start=True, stop=True)
                nc.scalar.activation(out=skip_sb[:, g * HD:(g + 1) * HD],
                                     in_=ps[:], func=ACTF.Copy)

            agg_sb = singles.tile([P, ngrp * HD], F32)
            den_sb = singles.tile([P, ngrp * H], F32)
            eps_t = singles.tile([P, 1], F32)
            nc.vector.memset(eps_t[:], 1e-5)

            # ---- edge phase: per merged run (R equal-K groups), gather the
            # nodes' contiguous neighbor blocks (one descriptor per node)
            # and reduce. All APs stay within 3 free dims.
            for (g0r, g1r, K) in runs:
                R = g1r - g0r
                RK = R * K
                r0 = int(colbase[g0r]) * P
                in_ap = tab_d[r0:r0 + RK * P, :].rearrange(
                    "(n k) e -> n (k e)", k=K)
                qv_t = qvp.tile([P, mrc * 2 * HD], F32, tag="qv")
                nc.gpsimd.dma_gather(
                    out_ap=qv_t[:, :RK * 2 * HD].rearrange(
                        "p (c e) -> p c e", e=K * 2 * HD),
                    in_ap=in_ap,
                    idxs_ap=idx_sb[:, :8 * R],
                    num_idxs=P * R,
                    num_idxs_reg=P * R,
                    elem_size=K * 2 * HD,
                    single_packet=False,
                )
                c0g = int(colbase[g0r])
                qv0 = qv_t[:, 0:1]
                pp = qv0.ap[0]

                # score: a[p, rk, h] = sum_d q[p,rk,h,d] * kdst[p,r,h,d]
                q3 = bass.AP(tensor=qv0.tensor, offset=qv0.offset,
                             ap=[pp, [2 * HD * K, R], [2 * HD, K], [1, HD]])
                kk = k_sb[:, g0r * HD:g1r * HD]
                kb = bass.AP(tensor=kk.tensor, offset=kk.offset,
                             ap=[kk.ap[0], [HD, R], [0, K], [1, HD]])
                prod = scr.tile([P, mrc * HD], F32, tag="scr")
                pv = prod[:, :RK * HD]
                p3 = bass.AP(tensor=pv.tensor, offset=pv.offset,
                             ap=[pv.ap[0], [K * HD, R], [HD, K], [1, HD]])
                nc.vector.tensor_tensor(out=p3, in0=q3, in1=kb, op=ALU.mult)

                ex = exs.tile([P, max(mrc, 2 * ngrp // H + 2) * H], F32,
                              tag="ex")
                e3 = bass.AP(tensor=ex[:].tensor, offset=ex[:].offset,
                             ap=[ex[:].ap[0], [H, RK], [1, H]])
                p3r = bass.AP(tensor=pv.tensor, offset=pv.offset,
                              ap=[pv.ap[0], [HD, RK], [D, H], [1, D]])
                nc.vector.tensor_reduce(out=e3, in_=p3r, axis=AX.X,
                                        op=ALU.add)
                # ex = exp(a/4) * mask
                exf = ex[:, :RK * H]
                nc.scalar.activation(out=exf, in_=exf, func=ACTF.Exp,
                                     scale=0.25)
                mm = mask_sb[:, c0g:c0g + RK]
                mb = bass.AP(tensor=mm.tensor, offset=mm.offset,
                             ap=[mm.ap[0], [1, RK], [0, H]])
                e2 = bass.AP(tensor=exf.tensor, offset=exf.offset,
                             ap=[exf.ap[0], [H, RK], [1, H]])
                nc.vector.tensor_tensor(out=e2, in0=e2, in1=mb, op=ALU.mult)
                # denom[p, r, h] = sum_k ex
                dd = den_sb[:, g0r * H:g1r * H]
                e4 = bass.AP(tensor=exf.tensor, offset=exf.offset,
                             ap=[exf.ap[0], [K * H, R], [1, H], [H, K]])
                nc.vector.tensor_reduce(out=dd, in_=e4, axis=AX.X,
                                        op=ALU.add)
                # w[p, rk, h, d] = v * ex ; agg[p, r, hd] = sum_k w
                v3 = bass.AP(tensor=qv0.tensor, offset=qv0.offset + HD,
                             ap=[pp, [2 * HD, RK], [D, H], [1, D]])
                eb = bass.AP(tensor=exf.tensor, offset=exf.offset,
                             ap=[exf.ap[0], [H, RK], [1, H], [0, D]])
                w_t = scr.tile([P, mrc * HD], F32, tag="scr")
                wv = w_t[:, :RK * HD]
                w3 = bass.AP(tensor=wv.tensor, offset=wv.offset,
                             ap=[wv.ap[0], [HD, RK], [D, H], [1, D]])
                nc.vector.tensor_tensor(out=w3, in0=v3, in1=eb, op=ALU.mult)
                aa = agg_sb[:, g0r * HD:g1r * HD]
                wr = bass.AP(tensor=wv.tensor, offset=wv.offset,
                             ap=[wv.ap[0], [K * HD, R], [1, HD], [HD, K]])
                nc.vector.tensor_reduce(out=aa, in_=wr, axis=AX.X,
                                        op=ALU.add)

            # ---- node phase
            NG = ngrp
            # dinv = 1 / (den + 1e-9)
            nc.vector.tensor_scalar(out=den_sb[:], in0=den_sb[:],
                                    scalar1=1e-9, scalar2=None, op0=ALU.add)
            nc.vector.reciprocal(out=den_sb[:], in_=den_sb[:])
            # rst = agg * dinv (broadcast over d)
            rst = agg_sb
            din = den_sb[:]
            dinb = bass.AP(tensor=din.tensor, offset=din.offset,
                           ap=[din.ap[0], [1, NG * H], [0, D]])
            r3 = rst[:].rearrange("p (c d) -> p c d", d=D)
            nc.vector.tensor_tensor(out=r3, in0=r3, in1=dinb, op=ALU.mult)

            # gate logits
            z_t = singles.tile([P, ngrp * HD], F32)
            gl = exs.tile([P, max(mrc, 2 * ngrp // H + 2) * H], F32, tag="ex")
            wg1b = bass.AP(tensor=wg1.tensor, offset=wg1.offset,
                           ap=[wg1.ap[0], [0, NG], [1, HD]])
            wg2b = bass.AP(tensor=wg2.tensor, offset=wg2.offset,
                           ap=[wg2.ap[0], [0, NG], [1, HD]])
            zv = z_t[:, :NG * HD].rearrange("p (c f) -> p c f", f=HD)
            nc.vector.tensor_tensor(out=zv, in0=skip_sb[:].rearrange(
                "p (c f) -> p c f", f=HD), in1=wg1b, op=ALU.mult)
            nc.vector.tensor_reduce(out=gl[:, 0:NG], in_=zv, axis=AX.X,
                                    op=ALU.add)
            nc.gpsimd.tensor_tensor(out=zv, in0=rst[:].rearrange(
                "p (c f) -> p c f", f=HD), in1=wg2b, op=ALU.mult)
            nc.vector.tensor_reduce(out=gl[:, NG:2 * NG], in_=zv, axis=AX.X,
                                    op=ALU.add)
            nc.vector.tensor_tensor(out=gl[:, 0:NG], in0=gl[:, 0:NG],
                                    in1=gl[:, NG:2 * NG], op=ALU.add)
            nc.scalar.activation(out=gl[:, 0:NG], in_=gl[:, 0:NG],
                                 func=ACTF.Sigmoid, bias=bg)
            # rst = rst + gate * (skip - rst)
            dif = z_t[:, :NG * HD]
            nc.vector.tensor_tensor(out=dif, in0=skip_sb[:], in1=rst[:],
                                    op=ALU.subtract)
            gv = gl[:, 0:NG]
            gb_ = bass.AP(tensor=gv.tensor, offset=gv.offset,
                          ap=[gv.ap[0], [1, NG], [0, HD]])
            d3v = dif.rearrange("p (c f) -> p c f", f=HD)
            nc.vector.tensor_tensor(out=d3v, in0=d3v, in1=gb_, op=ALU.mult)
            nc.gpsimd.tensor_tensor(out=rst[:], in0=rst[:], in1=dif,
                                    op=ALU.add)

            # LayerNorm
            mu = exs.tile([P, max(mrc, 2 * ngrp // H + 2) * H], F32, tag="ex")
            r3f = rst[:].rearrange("p (c f) -> p c f", f=HD)
            nc.vector.tensor_reduce(out=mu[:, 0:NG], in_=r3f, axis=AX.X,
                                    op=ALU.add)
            nc.vector.tensor_scalar(out=mu[:, 0:NG], in0=mu[:, 0:NG],
                                    scalar1=1.0 / HD, scalar2=None,
                                    op0=ALU.mult)
            mub = bass.AP(tensor=mu[:].tensor, offset=mu[:].offset,
                          ap=[mu[:].ap[0], [1, NG], [0, HD]])
            nc.vector.tensor_tensor(out=r3f, in0=r3f, in1=mub, op=ALU.subtract)
            sq = z_t[:, :NG * HD]
            nc.gpsimd.tensor_tensor(out=sq, in0=rst[:], in1=rst[:],
                                    op=ALU.mult)
            vs = mu[:, NG:2 * NG]
            nc.vector.tensor_reduce(out=vs, in_=sq.rearrange(
                "p (c f) -> p c f", f=HD), axis=AX.X, op=ALU.add)
            nc.scalar.activation(out=vs, in_=vs, func=ACTF.Sqrt,
                                 scale=1.0 / HD, bias=eps_t[:])
            nc.vector.reciprocal(out=vs, in_=vs)
            vsb = bass.AP(tensor=vs.tensor, offset=vs.offset,
                          ap=[vs.ap[0], [1, NG], [0, HD]])
            nc.vector.tensor_tensor(out=r3f, in0=r3f, in1=vsb, op=ALU.mult)
            gammab = bass.AP(tensor=gamma.tensor, offset=gamma.offset,
                             ap=[gamma.ap[0], [0, NG], [1, HD]])
            nc.vector.tensor_tensor(out=r3f, in0=r3f, in1=gammab, op=ALU.mult)
            betab = bass.AP(tensor=beta.tensor, offset=beta.offset,
                            ap=[beta.ap[0], [0, NG], [1, HD]])
            nc.gpsimd.tensor_tensor(out=r3f, in0=r3f, in1=betab, op=ALU.add)
            # prelu: max(x,0) + a*min(x,0)
            pos = z_t[:, :NG * HD]
            nc.vector.tensor_scalar(out=pos, in0=rst[:], scalar1=0.0,
                                    scalar2=None, op0=ALU.max)
            nc.vector.tensor_scalar(out=rst[:], in0=rst[:], scalar1=0.0,
                                    scalar2=None, op0=ALU.min)
            nc.vector.scalar_tensor_tensor(out=rst[:], in0=rst[:], scalar=pa,
                                           in1=pos, op0=ALU.mult, op1=ALU.add)
            nc.sync.dma_start(out=out_d[:], in_=rst[:])

    nc.compile()
    return nc


# ------------------------------------------------------------------- driver

_CACHE = {}


def _get_nc(plan, ncores):
    key = (tuple(plan["K"].tolist()), plan["grid"], plan["totc"], ncores)
    if key not in _CACHE:
        _CACHE[key] = _build_nc(plan, ncores)
    return _CACHE[key]


def _make_inmaps(plan, params, ncores):
    (Wk, bk, Wskip, bskip, Wgate, bgate, ln_gamma, ln_beta, prelu_a) = params
    wk = np.concatenate([np.asarray(Wk, np.float32),
                         np.asarray(bk, np.float32).reshape(1, HD)])
    wsk = np.concatenate([np.asarray(Wskip, np.float32),
                          np.asarray(bskip, np.float32).reshape(1, HD)])
    wg = np.asarray(Wgate, np.float32).reshape(3 * HD)
    par = np.zeros((1, 258), np.float32)
    par[0, 0:64] = wg[0:64] + wg[128:192]        # acts on skip
    par[0, 64:128] = wg[64:128] - wg[128:192]    # acts on rst
    par[0, 128] = np.float32(np.asarray(bgate).reshape(-1)[0])
    par[0, 129:193] = np.asarray(ln_gamma, np.float32)
    par[0, 193:257] = np.asarray(ln_beta, np.float32)
    par[0, 257] = np.float32(np.asarray(prelu_a).reshape(-1)[0])

    in_maps = []
    for c in range(ncores):
        pc = plan["per_core"][c]
        m = dict(featT=plan["featTs"][c], idx=plan["idx_w"], mask=pc["mask"],
                 tab=pc["tab"], wk=wk, wsk=wsk, par=par)
        in_maps.append(m)
    return in_maps


def run(q_src, v_src, feat, src, dst, Wk, bk, Wskip, bskip, Wgate, bgate,
        ln_gamma, ln_beta, prelu_a, ncores=NCORES, trace=False):
    plan = _plan(q_src, v_src, feat, src, dst, ncores)
    nc = _get_nc(plan, ncores)
    in_maps = _make_inmaps(
        plan, (Wk, bk, Wskip, bskip, Wgate, bgate, ln_gamma, ln_beta, prelu_a),
        ncores)
    res = run_bass_kernel_spmd(nc, in_maps, core_ids=list(range(ncores)),
                               trace=trace)
    n, npc, ngrp = plan["n"], plan["npc"], plan["ngrp"]
    out = np.empty((n, HD), np.float32)
    for c in range(ncores):
        r = res.results[c]["out"]                          # [128, ngrp*64]
        arr = r.reshape(P, ngrp, HD).transpose(1, 0, 2).reshape(-1, HD)
        out[c * npc + plan["cores"][c]["perm"]] = arr[plan["ndum"]:plan["ndum"] + npc]
    return out, res, plan, in_maps, nc


def kernel(**inputs):
    out, _, _, _, _ = run(**inputs)
    return out



# revision 3
# speedup vs baseline: 2.0294x; 2.0294x over previous
"""Trainium2 Bass kernel for nn_DenTargetTransformerConv (GNN message passing).

Strategy (graph/data parallel, dst-owner sharding across 8 NeuronCores):
  - Nodes are partitioned by dst-id range; each core owns N/8 nodes and all
    edges whose dst falls in its range.  The halo exchange of src features is
    materialized host-side as a per-core edge-expanded bf16 table (one
    256-byte [q|v] row per edge slot, rows laid out in consumption order), so
    the device reads it with plain strided hardware-DGE DMAs at full
    bandwidth - no gathers, no gpsimd.
  - Per core, own nodes are sorted by in-degree and packed into groups of
    128 (SBUF partition dim).  Consecutive groups are merged into runs of
    R <= 7 groups sharing a padded slot count Kp (R*Kp <= 96); all 8 cores
    share one compiled program.
  - All edge-phase math is bf16 with (d,h)-minor layouts so every
    tensor_tensor hits the DVE 2x mode.  The two big reductions (score
    over d, aggregation over k) run on the tensor engine as identity-matmul
    slice accumulations into PSUM (2.4 GHz, overlapped with DVE); the
    softmax denominator rides along in the same accumulation as 4 extra
    columns.  Edge-softmax max-subtraction is elided (scores are O(+-5)).
    Padded slots contribute exp(0)=1 to the denominator; a host-computed
    per-node correction (deg - Kp) fixes it up.
  - Node phase (gate + LayerNorm + PReLU) is bf16 on DVE with the
    broadcast-heavy 1x multiplies offloaded to gpsimd and the activations
    (sigmoid / square / sqrt / prelu) on the scalar engine.
"""

import numpy as np
import ml_dtypes

import concourse.bacc as bacc
import concourse.bass as bass
import concourse.tile as tile
from concourse import mybir
from concourse.bass_utils import run_bass_kernel_spmd

F32 = mybir.dt.float32
BF16 = mybir.dt.bfloat16
BF = ml_dtypes.bfloat16
AX = mybir.AxisListType
ALU = mybir.AluOpType
ACTF = mybir.ActivationFunctionType

P = 128
NCORES = 8
HD = 64          # H * D
H, D = 4, 16
IN_F = 64

RMAX = 7         # groups per run  (R*68 <= 476 PSUM f32 bank)
RKMAX = 96       # slot-columns per run (SBUF tiles)


# ----------------------------------------------------------------- host prep

def _plan(q_src, v_src, feat, src, dst, ncores):
    n = feat.shape[0]
    npc = n // ncores
    ngrp = (npc + P - 1) // P
    grid = ngrp * P
    ndum = grid - npc

    # per-node [q | v] rows in (d, h)-minor order, bf16
    q2 = np.asarray(q_src, np.float32).reshape(n, H, D).transpose(0, 2, 1)
    v2 = np.asarray(v_src, np.float32).reshape(n, H, D).transpose(0, 2, 1)
    qv = np.concatenate([q2.reshape(n, HD), v2.reshape(n, HD)],
                        axis=1).astype(BF)          # [n, 128]

    src = np.asarray(src).astype(np.int64)
    dst = np.asarray(dst).astype(np.int64)
    order = np.argsort(dst, kind="stable")
    dst_s, src_s = dst[order], src[order]
    bounds = np.searchsorted(dst_s, np.arange(ncores + 1) * npc)

    cores = []
    gmax = np.zeros((ncores, ngrp), np.int64)
    gdegs = []
    for c in range(ncores):
        lo, hi = bounds[c], bounds[c + 1]
        dstL = dst_s[lo:hi] - c * npc          # ascending
        srcL = src_s[lo:hi]
        deg = np.bincount(dstL, minlength=npc)
        starts = np.concatenate([[0], np.cumsum(deg)])
        rank = np.arange(len(dstL)) - starts[dstL]
        perm = np.argsort(deg, kind="stable")  # ascending degree
        pos_of = np.empty(npc, np.int64)
        pos_of[perm] = ndum + np.arange(npc)
        gd = np.zeros(grid, np.int64)
        gd[ndum:] = deg[perm]
        gdeg = gd.reshape(ngrp, P)             # [g, p]
        gmax[c] = gdeg.max(1)
        gdegs.append(gdeg)
        cores.append(dict(dstL=dstL, srcL=srcL, rank=rank, perm=perm,
                          pos_of=pos_of))

    K = np.maximum(gmax.max(0), 1)             # shared per-group slot count

    # runs of consecutive groups, padded to the run max Kp
    runs = []       # (g0, g1, Kp)
    g = 0
    while g < ngrp:
        ge = g + 1
        while (ge < ngrp and ge - g < RMAX
               and (ge - g + 1) * K[ge] <= RKMAX
               and K[ge] <= K[g] * 1.3 + 2):
            ge += 1
        runs.append((g, ge, int(K[ge - 1])))
        g = ge
    Kpad = np.zeros(ngrp, np.int64)
    for (g0, g1, kp) in runs:
        Kpad[g0:g1] = kp
    assert Kpad.max() <= RKMAX

    colbase = np.concatenate([[0], np.cumsum(Kpad)]).astype(np.int64)
    totc = int(colbase[-1])

    # per-core tab + denominator correction
    per_core = []
    for c in range(ncores):
        cd = cores[c]
        pos_e = cd["pos_of"][cd["dstL"]]       # grid position of each edge
        g_e = pos_e // P
        p_e = pos_e % P
        tab = np.zeros((totc * P, 2 * HD), BF)
        rows = colbase[g_e] * P + p_e * Kpad[g_e] + cd["rank"]
        tab[rows] = qv[cd["srcL"]]
        corr = (gdegs[c].T.astype(np.float32) -
                Kpad[None, :].astype(np.float32)) + 1e-9       # [P, ngrp]
        corr_w = np.repeat(corr, H, axis=1).astype(np.float32)  # [P, ngrp*H]
        per_core.append(dict(tab=tab, corr=corr_w))

    # featT with ones row, per core, grid-permuted: [IN_F+1, grid] bf16
    featTs = []
    feat = np.asarray(feat, np.float32)
    for c in range(ncores):
        ft = np.zeros((IN_F + 1, grid), np.float32)
        ft[IN_F, :] = 1.0
        perm = cores[c]["perm"]
        ft[:IN_F, ndum:] = feat[c * npc + perm].T
        featTs.append(ft.astype(BF))

    ident = np.eye(P, dtype=BF)

    return dict(n=n, npc=npc, ngrp=ngrp, grid=grid, ndum=ndum, K=K,
                Kpad=Kpad, colbase=colbase, totc=totc, runs=runs,
                ident=ident, cores=cores, per_core=per_core, featTs=featTs)


# standard (h,d) column index -> (d,h) position
def _dh_perm():
    cm = np.empty(HD, np.int64)
    for d in range(D):
        for h in range(H):
            cm[d * H + h] = h * D + d
    return cm            # newcol j takes oldcol cm[j]


# ------------------------------------------------------------- device build

def _build_nc(plan, ncores):
    ngrp, runs = plan["ngrp"], plan["runs"]
    grid = plan["grid"]
    colbase = plan["colbase"]
    NG = ngrp

    nc = bacc.Bacc("TRN2", target_bir_lowering=False, debug=False,
                   num_devices=ncores)

    featT_d = nc.dram_tensor("featT", [IN_F + 1, grid], BF16,
                             kind="ExternalInput").ap()
    wkws_d = nc.dram_tensor("wkws", [IN_F + 1, 2 * HD], BF16,
                            kind="ExternalInput").ap()
    tab_d = nc.dram_tensor("tab", [plan["totc"] * P, 2 * HD], BF16,
                           kind="ExternalInput").ap()
    corr_d = nc.dram_tensor("corr", [P, ngrp * H], F32,
                            kind="ExternalInput").ap()
    ident_d = nc.dram_tensor("ident", [P, P], BF16, kind="ExternalInput").ap()
    # bf16 params row: [wg1'(64) | wg2'(64) | gamma(64) | beta(64)]
    parb_d = nc.dram_tensor("parb", [1, 4 * HD], BF16,
                            kind="ExternalInput").ap()
    # f32 params row: [bgate, prelu_a, ln_eps]
    parf_d = nc.dram_tensor("parf", [1, 3], F32, kind="ExternalInput").ap()
    out_d = nc.dram_tensor("out", [P, ngrp * HD], BF16,
                           kind="ExternalOutput").ap()

    with tile.TileContext(nc) as tc:
        with (
            tc.tile_pool(name="singles", bufs=1) as singles,
            tc.tile_pool(name="psL", bufs=2, space="PSUM") as psL,
            tc.tile_pool(name="psS", bufs=2, space="PSUM") as psS,
            tc.tile_pool(name="psA", bufs=2, space="PSUM") as psA,
            tc.tile_pool(name="qvp", bufs=2) as qvp,
            tc.tile_pool(name="prodp", bufs=2) as prodp,
            tc.tile_pool(name="wp", bufs=2) as wp,
        ):
            # ---- static loads
            featT = singles.tile([IN_F + 1, grid], BF16)
            nc.sync.dma_start(out=featT[:], in_=featT_d[:])
            wkws = singles.tile([IN_F + 1, 2 * HD], BF16)
            nc.sync.dma_start(out=wkws[:], in_=wkws_d[:])
            ident = singles.tile([P, P], BF16)
            nc.sync.dma_start(out=ident[:], in_=ident_d[:])
            corr_sb = singles.tile([P, ngrp * H], F32)
            nc.sync.dma_start(out=corr_sb[:], in_=corr_d[:])
            parb = singles.tile([P, 4 * HD], BF16)
            nc.gpsimd.dma_start(
                out=parb[:],
                in_=bass.AP(tensor=parb_d.tensor, offset=parb_d.offset,
                            ap=[[0, P], [1, 4 * HD]]))
            parf = singles.tile([P, 3], F32)
            nc.gpsimd.dma_start(
                out=parf[:],
                in_=bass.AP(tensor=parf_d.tensor, offset=parf_d.offset,
                            ap=[[0, P], [1, 3]]))
            bg = parf[:, 0:1]
            pa = parf[:, 1:2]
            eps = parf[:, 2:3]

            # ---- per-node linears on PE: ks[:, g*128 : ...] = [k | skip]
            ks_sb = singles.tile([P, ngrp * 2 * HD], BF16)
            for q0 in range(0, ngrp, 4):
                q1 = min(q0 + 4, ngrp)
                pl = psL.tile([P, 512], F32, tag="pl")
                for g in range(q0, q1):
                    nc.tensor.matmul(out=pl[:, (g - q0) * 128:(g - q0 + 1) * 128],
                                     lhsT=featT[:, g * P:(g + 1) * P],
                                     rhs=wkws[:], start=True, stop=True)
                nc.scalar.activation(
                    out=ks_sb[:, q0 * 128:q1 * 128],
                    in_=pl[:, :(q1 - q0) * 128], func=ACTF.Copy)

            agg_sb = singles.tile([P, ngrp * HD], BF16)
            den_sb = singles.tile([P, ngrp * H], F32)

            # ---- edge phase
            for ri, (g0, g1, K) in enumerate(runs):
                R = g1 - g0
                RK = R * K
                r0 = int(colbase[g0]) * P
                qv_t = qvp.tile([P, RKMAX * 2 * HD], BF16, tag="qv")
                in_ap = bass.AP(
                    tensor=tab_d.tensor,
                    offset=tab_d.offset + r0 * 2 * HD,
                    ap=[[K * 2 * HD, P], [P * K * 2 * HD, R],
                        [1, K * 2 * HD]])
                dmaeng = nc.sync if ri % 2 == 0 else nc.scalar
                dmaeng.dma_start(out=qv_t[:, :RK * 2 * HD], in_=in_ap)

                qv0 = qv_t[:, 0:1]
                pp = qv0.ap[0]

                # prod[p, rk, (d h)] = q[p, rk, :] * k_dst[p, r, :]
                prod = prodp.tile([P, RKMAX * HD], BF16, tag="prod")
                pv = prod[:, :RK * HD]
                p2 = bass.AP(tensor=pv.tensor, offset=pv.offset,
                             ap=[pv.ap[0], [HD, RK], [1, HD]])
                q2 = bass.AP(tensor=qv0.tensor, offset=qv0.offset,
                             ap=[pp, [2 * HD, RK], [1, HD]])
                kk = ks_sb[:, 0:1]
                kb = bass.AP(tensor=kk.tensor, offset=kk.offset + g0 * 2 * HD,
                             ap=[kk.ap[0], [2 * HD, R], [0, K], [1, HD]])
                nc.vector.tensor_tensor(out=p2, in0=q2, in1=kb, op=ALU.mult)

                # score[p, rk, h] = sum_d prod : 16 identity matmuls into PSUM
                ps = psS.tile([P, RKMAX * H], F32, tag="ps")
                sv = ps[:, :RK * H]
                for d in range(D):
                    rhs = bass.AP(tensor=pv.tensor, offset=pv.offset + d * H,
                                  ap=[pv.ap[0], [HD, RK], [1, H]])
                    nc.tensor.matmul(out=sv, lhsT=ident[:], rhs=rhs,
                                     start=(d == 0), stop=(d == D - 1))

                # w tile rows of 68: [w(64) | ex(4)] per slot
                w_t = wp.tile([P, RKMAX * 68], BF16, tag="w")
                wv = w_t[:, 0:1]
                exo = bass.AP(tensor=wv.tensor, offset=wv.offset + HD,
                              ap=[wv.ap[0], [68 * K, R], [68, K], [1, H]])
                nc.scalar.activation(out=exo, in_=sv, func=ACTF.Exp,
                                     scale=0.25)

                # w[p, r, k, (d h)] = v * ex  (per group r: 3-dim APs)
                for r in range(R):
                    wo = bass.AP(tensor=wv.tensor, offset=wv.offset + r * K * 68,
                                 ap=[wv.ap[0], [68, K], [1, HD]])
                    vo = bass.AP(tensor=qv0.tensor,
                                 offset=qv0.offset + r * K * 2 * HD + HD,
                                 ap=[pp, [2 * HD, K], [1, HD]])
                    eb = bass.AP(tensor=wv.tensor,
                                 offset=wv.offset + r * K * 68 + HD,
                                 ap=[wv.ap[0], [68, K], [0, D], [1, H]])
                    nc.vector.tensor_tensor(out=wo, in0=vo, in1=eb,
                                            op=ALU.mult)

                # agg[p, r, 68] = sum_k w-rows : K identity matmuls into PSUM
                pa_t = psA.tile([P, RMAX * 68], F32, tag="pa")
                av = pa_t[:, :R * 68]
                for k in range(K):
                    rhs = bass.AP(tensor=wv.tensor, offset=wv.offset + k * 68,
                                  ap=[wv.ap[0], [68 * K, R], [1, 68]])
                    nc.tensor.matmul(out=av, lhsT=ident[:], rhs=rhs,
                                     start=(k == 0), stop=(k == K - 1))

                # copy agg/den out of PSUM
                nc.scalar.activation(
                    out=bass.AP(tensor=agg_sb[:].tensor,
                                offset=agg_sb[:].offset + g0 * HD,
                                ap=[agg_sb[:].ap[0], [HD, R], [1, HD]]),
                    in_=bass.AP(tensor=av.tensor, offset=av.offset,
                                ap=[av.ap[0], [68, R], [1, HD]]),
                    func=ACTF.Copy)
                nc.scalar.activation(
                    out=bass.AP(tensor=den_sb[:].tensor,
                                offset=den_sb[:].offset + g0 * H,
                                ap=[den_sb[:].ap[0], [H, R], [1, H]]),
                    in_=bass.AP(tensor=av.tensor, offset=av.offset + HD,
                                ap=[av.ap[0], [68, R], [1, H]]),
                    func=ACTF.Copy)

            # ---- node phase
            # dinv = 1 / (den + (deg - Kp) + 1e-9), as bf16
            nc.vector.tensor_tensor(out=den_sb[:], in0=den_sb[:],
                                    in1=corr_sb[:], op=ALU.add)
            nc.vector.reciprocal(out=den_sb[:], in_=den_sb[:])
            dinv = singles.tile([P, ngrp * H], BF16)
            nc.vector.tensor_scalar(out=dinv[:], in0=den_sb[:],
                                    scalar1=1.0, scalar2=None, op0=ALU.mult)

            # rst = agg * dinv  (broadcast over d)
            rst = singles.tile([P, ngrp * HD], BF16)
            r3 = rst[:].rearrange("p (c e) -> p c e", e=HD)
            a3 = agg_sb[:].rearrange("p (c e) -> p c e", e=HD)
            dv = dinv[:, 0:1]
            dinb = bass.AP(tensor=dv.tensor, offset=dv.offset,
                           ap=[dv.ap[0], [H, NG], [0, D], [1, H]])
            nc.vector.tensor_tensor(out=r3, in0=a3, in1=dinb, op=ALU.mult)

            ksv = ks_sb[:, 0:1]
            skipb = bass.AP(tensor=ksv.tensor, offset=ksv.offset + HD,
                            ap=[ksv.ap[0], [2 * HD, NG], [1, HD]])
            pb = parb[:, 0:1]
            wg1b = bass.AP(tensor=pb.tensor, offset=pb.offset,
                           ap=[pb.ap[0], [0, NG], [1, HD]])
            wg2b = bass.AP(tensor=pb.tensor, offset=pb.offset + HD,
                           ap=[pb.ap[0], [0, NG], [1, HD]])
            gammab = bass.AP(tensor=pb.tensor, offset=pb.offset + 2 * HD,
                             ap=[pb.ap[0], [0, NG], [1, HD]])
            betab = bass.AP(tensor=pb.tensor, offset=pb.offset + 3 * HD,
                            ap=[pb.ap[0], [0, NG], [1, HD]])

            # gate logits
            z_t = singles.tile([P, ngrp * HD], BF16)
            z3 = z_t[:].rearrange("p (c e) -> p c e", e=HD)
            gl = singles.tile([P, 2 * ngrp], F32)
            nc.vector.tensor_tensor(out=z3, in0=skipb, in1=wg1b, op=ALU.mult)
            nc.vector.tensor_reduce(out=gl[:, 0:NG], in_=z3, axis=AX.X,
                                    op=ALU.add)
            nc.vector.tensor_tensor(out=z3, in0=r3, in1=wg2b, op=ALU.mult)
            nc.vector.tensor_reduce(out=gl[:, NG:2 * NG], in_=z3, axis=AX.X,
                                    op=ALU.add)
            nc.vector.tensor_tensor(out=gl[:, 0:NG], in0=gl[:, 0:NG],
                                    in1=gl[:, NG:2 * NG], op=ALU.add)
            gate = singles.tile([P, ngrp], BF16)
            nc.scalar.activation(out=gate[:], in_=gl[:, 0:NG],
                                 func=ACTF.Sigmoid, bias=bg)

            # rst = rst + gate * (skip - rst)
            dif = singles.tile([P, ngrp * HD], BF16)
            d3 = dif[:].rearrange("p (c e) -> p c e", e=HD)
            nc.vector.tensor_tensor(out=d3, in0=skipb, in1=r3, op=ALU.subtract)
            gv = gate[:, 0:1]
            gb = bass.AP(tensor=gv.tensor, offset=gv.offset,
                         ap=[gv.ap[0], [1, NG], [0, HD]])
            nc.gpsimd.tensor_tensor(out=d3, in0=d3, in1=gb, op=ALU.mult)
            nc.vector.tensor_tensor(out=rst[:], in0=rst[:], in1=dif[:],
                                    op=ALU.add)

            # LayerNorm
            mu = singles.tile([P, 2 * ngrp], F32)
            nc.vector.tensor_reduce(out=mu[:, 0:NG], in_=r3, axis=AX.X,
                                    op=ALU.add)
            muv = mu[:, 0:1]
            mub = bass.AP(tensor=muv.tensor, offset=muv.offset,
                          ap=[muv.ap[0], [1, NG], [0, HD]])
            nc.vector.scalar_tensor_tensor(out=rst[:], in0=mub,
                                           scalar=-1.0 / HD, in1=rst[:],
                                           op0=ALU.mult, op1=ALU.add)
            sq = z_t  # reuse
            nc.scalar.activation(out=sq[:], in_=rst[:], func=ACTF.Square)
            nc.vector.tensor_reduce(out=mu[:, NG:2 * NG],
                                    in_=sq[:].rearrange("p (c e) -> p c e",
                                                        e=HD),
                                    axis=AX.X, op=ALU.add)
            nc.scalar.activation(out=mu[:, NG:2 * NG], in_=mu[:, NG:2 * NG],
                                 func=ACTF.Sqrt, scale=1.0 / HD, bias=eps)
            nc.vector.reciprocal(out=mu[:, NG:2 * NG], in_=mu[:, NG:2 * NG])
            isd = singles.tile([P, ngrp], BF16)
            nc.vector.tensor_scalar(out=isd[:], in0=mu[:, NG:2 * NG],
                                    scalar1=1.0, scalar2=None, op0=ALU.mult)
            iv = isd[:, 0:1]
            isb = bass.AP(tensor=iv.tensor, offset=iv.offset,
                          ap=[iv.ap[0], [1, NG], [0, HD]])
            nc.gpsimd.tensor_tensor(out=rst[:], in0=rst[:], in1=isb,
                                    op=ALU.mult)
            nc.vector.tensor_tensor(out=r3, in0=r3, in1=gammab, op=ALU.mult)
            nc.vector.tensor_tensor(out=r3, in0=r3, in1=betab, op=ALU.add)

            # PReLU: max(x,0) + a*min(x,0)
            pos = dif  # reuse
            nc.vector.tensor_scalar(out=pos[:], in0=rst[:], scalar1=0.0,
                                    scalar2=None, op0=ALU.max)
            nc.vector.tensor_scalar(out=rst[:], in0=rst[:], scalar1=0.0,
                                    scalar2=None, op0=ALU.min)
            nc.vector.scalar_tensor_tensor(out=rst[:], in0=rst[:], scalar=pa,
                                           in1=pos[:], op0=ALU.mult,
                                           op1=ALU.add)
            nc.sync.dma_start(out=out_d[:], in_=rst[:])

    nc.compile()
    return nc


# ------------------------------------------------------------------- driver

_CACHE = {}


def _get_nc(plan, ncores):
    key = (tuple(plan["Kpad"].tolist()), plan["grid"], ncores)
    if key not in _CACHE:
        _CACHE[key] = _build_nc(plan, ncores)
    return _CACHE[key]


def _make_inmaps(plan, params, ncores):
    (Wk, bk, Wskip, bskip, Wgate, bgate, ln_gamma, ln_beta, prelu_a) = params
    cm = _dh_perm()
    wk = np.concatenate([np.asarray(Wk, np.float32),
                         np.asarray(bk, np.float32).reshape(1, HD)])[:, cm]
    wsk = np.concatenate([np.asarray(Wskip, np.float32),
                          np.asarray(bskip, np.float32).reshape(1, HD)])[:, cm]
    wkws = np.concatenate([wk, wsk], axis=1).astype(BF)     # [65, 128]

    wg = np.asarray(Wgate, np.float32).reshape(3 * HD)
    parb = np.zeros((1, 4 * HD), np.float32)
    parb[0, 0:HD] = (wg[0:64] + wg[128:192])[cm]      # acts on skip
    parb[0, HD:2 * HD] = (wg[64:128] - wg[128:192])[cm]  # acts on rst
    parb[0, 2 * HD:3 * HD] = np.asarray(ln_gamma, np.float32)[cm]
    parb[0, 3 * HD:4 * HD] = np.asarray(ln_beta, np.float32)[cm]
    parb = parb.astype(BF)
    parf = np.array([[np.float32(np.asarray(bgate).reshape(-1)[0]),
                      np.float32(np.asarray(prelu_a).reshape(-1)[0]),
                      1e-5]], np.float32)

    in_maps = []
    for c in range(ncores):
        pc = plan["per_core"][c]
        m = dict(featT=plan["featTs"][c], tab=pc["tab"], corr=pc["corr"],
                 ident=plan["ident"], wkws=wkws, parb=parb, parf=parf)
        in_maps.append(m)
    return in_maps


def run(q_src, v_src, feat, src, dst, Wk, bk, Wskip, bskip, Wgate, bgate,
        ln_gamma, ln_beta, prelu_a, ncores=NCORES, trace=False):
    plan = _plan(q_src, v_src, feat, src, dst, ncores)
    nc = _get_nc(plan, ncores)
    in_maps = _make_inmaps(
        plan, (Wk, bk, Wskip, bskip, Wgate, bgate, ln_gamma, ln_beta, prelu_a),
        ncores)
    res = run_bass_kernel_spmd(nc, in_maps, core_ids=list(range(ncores)),
                               trace=trace)
    n, npc, ngrp = plan["n"], plan["npc"], plan["ngrp"]
    out = np.empty((n, HD), np.float32)
    for c in range(ncores):
        r = np.asarray(res.results[c]["out"]).astype(np.float32)
        # [P, ngrp, D, H] -> [ngrp, P, H, D] -> [grid, HD]
        arr = r.reshape(P, ngrp, D, H).transpose(1, 0, 3, 2).reshape(-1, HD)
        out[c * npc + plan["cores"][c]["perm"]] = \
            arr[plan["ndum"]:plan["ndum"] + npc]
    return out, res, plan, in_maps, nc


def kernel(**inputs):
    out, _, _, _, _ = run(**inputs)
    return out


# revision 4
# speedup vs baseline: 2.3623x; 1.1641x over previous
"""Trainium2 Bass kernel for nn_DenTargetTransformerConv (GNN message passing).

Strategy (graph/data parallel, dst-owner sharding across 8 NeuronCores):
  - Nodes are partitioned by dst-id range; each core owns N/8 nodes and all
    edges whose dst falls in its range.  The halo exchange of src features is
    materialized host-side as a per-core edge-expanded bf16 table (one
    256-byte [q|v] row per edge slot, rows laid out in consumption order), so
    the device reads it with plain strided hardware-DGE DMAs at full
    bandwidth - no gathers, no gpsimd.  Each run's table slice is fetched as
    two partition-halves, one on the SP HWDGE queue pool and one on the
    Activation pool, to spread load over all DMA engines.
  - Per core, own nodes are sorted by in-degree and packed into groups of
    128 (SBUF partition dim).  Consecutive groups are merged into runs of
    R <= 7 groups sharing a padded slot count Kp (R*Kp <= 96); all 8 cores
    share one compiled program.
  - All edge-phase math is bf16 with (d,h)-minor layouts so every
    tensor_tensor hits the DVE 2x mode.  The two big reductions (score
    over d, aggregation over k) run on the tensor engine as identity-matmul
    slice accumulations into PSUM (errata-free 2.4 GHz, overlapped with
    DVE); the softmax denominator rides along in the same accumulation as 4
    extra columns, and the skip-side gate logit rides along in the k/skip
    linear as a 129th column.  Edge-softmax max-subtraction is elided
    (scores are O(+-5)).  Padded slots contribute exp(0)=1 to the
    denominator; a host-computed per-node correction (deg - Kp) fixes it.
  - Emission is software-pipelined one stage deep (run i's DMA/prod/score
    before run i-1's w/agg/copyout) so no engine head-of-line blocks on a
    cross-engine dependency.  The node phase (gate + LayerNorm + PReLU) is
    processed in chunks of ~2 runs woven between edge runs, so only the
    last chunk's short serial chain sits after the final run.  Everything
    scalar-engine-side (exp, tanh-sigmoid, square, prelu, copies) lives in
    the one `exp_and_others` activation table - zero table reloads; rsqrt
    for LayerNorm is a quake-style bit hack + 2 Newton steps on tiny
    [128, G] tiles on DVE.
"""

import numpy as np
import ml_dtypes

import concourse.bacc as bacc
import concourse.bass as bass
import concourse.tile as tile
from concourse import mybir
from concourse.bass_utils import run_bass_kernel_spmd

F32 = mybir.dt.float32
I32 = mybir.dt.int32
BF16 = mybir.dt.bfloat16
BF = ml_dtypes.bfloat16
AX = mybir.AxisListType
ALU = mybir.AluOpType
ACTF = mybir.ActivationFunctionType

P = 128
NCORES = 8
HD = 64          # H * D
H, D = 4, 16
IN_F = 64

RMAX = 7         # groups per run  (R*68 <= 476 PSUM f32 bank)
RKMAX = 96       # slot-columns per run (SBUF tiles)
MAGIC = 0x5F3759DF


# ----------------------------------------------------------------- host prep

def _plan(q_src, v_src, feat, src, dst, ncores):
    n = feat.shape[0]
    npc = n // ncores
    ngrp = (npc + P - 1) // P
    grid = ngrp * P
    ndum = grid - npc

    # per-node [q | v] rows in (d, h)-minor order, bf16
    q2 = np.asarray(q_src, np.float32).reshape(n, H, D).transpose(0, 2, 1)
    v2 = np.asarray(v_src, np.float32).reshape(n, H, D).transpose(0, 2, 1)
    qv = np.concatenate([q2.reshape(n, HD), v2.reshape(n, HD)],
                        axis=1).astype(BF)          # [n, 128]

    src = np.asarray(src).astype(np.int64)
    dst = np.asarray(dst).astype(np.int64)
    order = np.argsort(dst, kind="stable")
    dst_s, src_s = dst[order], src[order]
    bounds = np.searchsorted(dst_s, np.arange(ncores + 1) * npc)

    cores = []
    gmax = np.zeros((ncores, ngrp), np.int64)
    gdegs = []
    for c in range(ncores):
        lo, hi = bounds[c], bounds[c + 1]
        dstL = dst_s[lo:hi] - c * npc          # ascending
        srcL = src_s[lo:hi]
        deg = np.bincount(dstL, minlength=npc)
        starts = np.concatenate([[0], np.cumsum(deg)])
        rank = np.arange(len(dstL)) - starts[dstL]
        perm = np.argsort(deg, kind="stable")  # ascending degree
        pos_of = np.empty(npc, np.int64)
        pos_of[perm] = ndum + np.arange(npc)
        gd = np.zeros(grid, np.int64)
        gd[ndum:] = deg[perm]
        gdeg = gd.reshape(ngrp, P)             # [g, p]
        gmax[c] = gdeg.max(1)
        gdegs.append(gdeg)
        cores.append(dict(dstL=dstL, srcL=srcL, rank=rank, perm=perm,
                          pos_of=pos_of))

    K = np.maximum(gmax.max(0), 1)             # shared per-group slot count

    # runs of consecutive groups, padded to the run max Kp
    runs = []       # (g0, g1, Kp)
    g = 0
    while g < ngrp:
        ge = g + 1
        while (ge < ngrp and ge - g < RMAX
               and (ge - g + 1) * K[ge] <= RKMAX
               and K[ge] <= K[g] * 1.3 + 2):
            ge += 1
        runs.append((g, ge, int(K[ge - 1])))
        g = ge
    Kpad = np.zeros(ngrp, np.int64)
    for (g0, g1, kp) in runs:
        Kpad[g0:g1] = kp
    assert Kpad.max() <= RKMAX

    colbase = np.concatenate([[0], np.cumsum(Kpad)]).astype(np.int64)
    totc = int(colbase[-1])

    # node-phase chunks: pairs of runs (absorb a trailing single run)
    chunks = []     # (run_lo, run_hi, g_lo, g_hi)
    i = 0
    while i < len(runs):
        j = min(i + 2, len(runs))
        if len(runs) - j == 1:
            j = len(runs)
        chunks.append((i, j, runs[i][0], runs[j - 1][1]))
        i = j

    # per-core tab + denominator correction
    per_core = []
    for c in range(ncores):
        cd = cores[c]
        pos_e = cd["pos_of"][cd["dstL"]]       # grid position of each edge
        g_e = pos_e // P
        p_e = pos_e % P
        tab = np.zeros((totc * P, 2 * HD), BF)
        rows = colbase[g_e] * P + p_e * Kpad[g_e] + cd["rank"]
        tab[rows] = qv[cd["srcL"]]
        corr = (gdegs[c].T.astype(np.float32) -
                Kpad[None, :].astype(np.float32)) + 1e-9       # [P, ngrp]
        corr_w = np.repeat(corr, H, axis=1).astype(np.float32)  # [P, ngrp*H]
        per_core.append(dict(tab=tab, corr=corr_w))

    # featT with ones row, per core, grid-permuted: [IN_F+1, grid] bf16
    featTs = []
    feat = np.asarray(feat, np.float32)
    for c in range(ncores):
        ft = np.zeros((IN_F + 1, grid), np.float32)
        ft[IN_F, :] = 1.0
        perm = cores[c]["perm"]
        ft[:IN_F, ndum:] = feat[c * npc + perm].T
        featTs.append(ft.astype(BF))

    ident = np.eye(P, dtype=BF)

    return dict(n=n, npc=npc, ngrp=ngrp, grid=grid, ndum=ndum, K=K,
                Kpad=Kpad, colbase=colbase, totc=totc, runs=runs,
                chunks=chunks, ident=ident, cores=cores, per_core=per_core,
                featTs=featTs)


# standard (h,d) column index -> (d,h) position
def _dh_perm():
    cm = np.empty(HD, np.int64)
    for d in range(D):
        for h in range(H):
            cm[d * H + h] = h * D + d
    return cm            # newcol j takes oldcol cm[j]


# ------------------------------------------------------------- device build

def _build_nc(plan, ncores):
    ngrp, runs, chunks = plan["ngrp"], plan["runs"], plan["chunks"]
    grid = plan["grid"]
    colbase = plan["colbase"]
    NG = ngrp
    nruns = len(runs)

    nc = bacc.Bacc("TRN2", target_bir_lowering=False, debug=False,
                   num_devices=ncores)

    featT_d = nc.dram_tensor("featT", [IN_F + 1, grid], BF16,
                             kind="ExternalInput").ap()
    wkws_d = nc.dram_tensor("wkws", [IN_F + 1, 132], BF16,
                            kind="ExternalInput").ap()
    tab_d = nc.dram_tensor("tab", [plan["totc"] * P, 2 * HD], BF16,
                           kind="ExternalInput").ap()
    corr_d = nc.dram_tensor("corr", [P, ngrp * H], F32,
                            kind="ExternalInput").ap()
    ident_d = nc.dram_tensor("ident", [P, P], BF16, kind="ExternalInput").ap()
    # bf16 params row: [wg2'(64) | gamma(64) | beta(64)]
    parb_d = nc.dram_tensor("parb", [1, 3 * HD], BF16,
                            kind="ExternalInput").ap()
    # f32 params row: [bgate/2, prelu_a, unused]
    parf_d = nc.dram_tensor("parf", [1, 3], F32, kind="ExternalInput").ap()
    out_d = nc.dram_tensor("out", [P, ngrp * HD], BF16,
                           kind="ExternalOutput").ap()

    with tile.TileContext(nc) as tc:
        with (
            tc.tile_pool(name="singles", bufs=1) as singles,
            tc.tile_pool(name="psL", bufs=2, space="PSUM") as psL,
            tc.tile_pool(name="psS", bufs=2, space="PSUM") as psS,
            tc.tile_pool(name="psA", bufs=2, space="PSUM") as psA,
            tc.tile_pool(name="qvp", bufs=3) as qvp,
            tc.tile_pool(name="prodp", bufs=2) as prodp,
            tc.tile_pool(name="wp", bufs=2) as wp,
            tc.tile_pool(name="nodep", bufs=2) as nodep,
        ):
            # ---- static loads
            featT = singles.tile([IN_F + 1, grid], BF16)
            nc.sync.dma_start(out=featT[:], in_=featT_d[:])
            wkws = singles.tile([IN_F + 1, 132], BF16)
            nc.sync.dma_start(out=wkws[:], in_=wkws_d[:])
            ident = singles.tile([P, P], BF16)
            nc.sync.dma_start(out=ident[:], in_=ident_d[:])
            corr_sb = singles.tile([P, ngrp * H], F32)
            nc.sync.dma_start(out=corr_sb[:], in_=corr_d[:])
            parb = singles.tile([P, 3 * HD], BF16)
            nc.gpsimd.dma_start(
                out=parb[:],
                in_=bass.AP(tensor=parb_d.tensor, offset=parb_d.offset,
                            ap=[[0, P], [1, 3 * HD]]))
            parf = singles.tile([P, 3], F32)
            nc.gpsimd.dma_start(
                out=parf[:],
                in_=bass.AP(tensor=parf_d.tensor, offset=parf_d.offset,
                            ap=[[0, P], [1, 3]]))
            bg2 = parf[:, 0:1]
            pa = parf[:, 1:2]

            # ---- per-node linears on PE: ks[:, g*128:...] = [k | skip],
            # plus the skip-side gate logit r1 as column 128.
            ks_sb = singles.tile([P, ngrp * 2 * HD], BF16)
            r1_sb = singles.tile([P, ngrp], F32)
            for q0 in range(0, ngrp, 3):
                q1 = min(q0 + 3, ngrp)
                nq = q1 - q0
                pl = psL.tile([P, 3 * 132], F32, tag="pl")
                for g in range(q0, q1):
                    nc.tensor.matmul(out=pl[:, (g - q0) * 132:(g - q0 + 1) * 132],
                                     lhsT=featT[:, g * P:(g + 1) * P],
                                     rhs=wkws[:], start=True, stop=True)
                plv = pl[:, 0:1]
                nc.scalar.activation(
                    out=bass.AP(tensor=ks_sb[:].tensor,
                                offset=ks_sb[:].offset + q0 * 2 * HD,
                                ap=[ks_sb[:].ap[0], [2 * HD, nq], [1, 2 * HD]]),
                    in_=bass.AP(tensor=plv.tensor, offset=plv.offset,
                                ap=[plv.ap[0], [132, nq], [1, 2 * HD]]),
                    func=ACTF.Copy)
                nc.scalar.activation(
                    out=r1_sb[:, q0:q1],
                    in_=bass.AP(tensor=plv.tensor, offset=plv.offset + 128,
                                ap=[plv.ap[0], [132, nq], [1, 1]]),
                    func=ACTF.Copy)

            agg_sb = singles.tile([P, ngrp * HD], BF16)
            den_sb = singles.tile([P, ngrp * H], F32)
            ksv = ks_sb[:, 0:1]
            pb = parb[:, 0:1]

            # ------------------------------------------------ stage emitters
            qv_tiles = {}
            prod_tiles = {}
            ps_tiles = {}
            w_tiles = {}
            pa_tiles = {}

            def s0_dma(i):
                (g0, g1, K) = runs[i]
                R = g1 - g0
                RK = R * K
                r0 = int(colbase[g0]) * P
                qv_t = qvp.tile([P, RKMAX * 2 * HD], BF16, tag="qv")
                qv_tiles[i] = qv_t
                half = 64 * K * 2 * HD
                for hi, eng in ((0, nc.sync), (1, nc.scalar)):
                    in_ap = bass.AP(
                        tensor=tab_d.tensor,
                        offset=tab_d.offset + r0 * 2 * HD + hi * half,
                        ap=[[K * 2 * HD, 64], [P * K * 2 * HD, R],
                            [1, K * 2 * HD]])
                    eng.dma_start(out=qv_t[hi * 64:(hi + 1) * 64,
                                           :RK * 2 * HD], in_=in_ap)

            def s1_prod(i):
                (g0, g1, K) = runs[i]
                R = g1 - g0
                RK = R * K
                qv0 = qv_tiles[i][:, 0:1]
                pp = qv0.ap[0]
                prod = prodp.tile([P, RKMAX * HD], BF16, tag="prod")
                prod_tiles[i] = prod
                pv = prod[:, :RK * HD]
                p2 = bass.AP(tensor=pv.tensor, offset=pv.offset,
                             ap=[pv.ap[0], [HD, RK], [1, HD]])
                q2 = bass.AP(tensor=qv0.tensor, offset=qv0.offset,
                             ap=[pp, [2 * HD, RK], [1, HD]])
                kb = bass.AP(tensor=ksv.tensor, offset=ksv.offset + g0 * 2 * HD,
                             ap=[ksv.ap[0], [2 * HD, R], [0, K], [1, HD]])
                nc.vector.tensor_tensor(out=p2, in0=q2, in1=kb, op=ALU.mult)

            def s2_score(i):
                (g0, g1, K) = runs[i]
                RK = (g1 - g0) * K
                pv = prod_tiles[i][:, :RK * HD]
                ps = psS.tile([P, RKMAX * H], F32, tag="ps")
                ps_tiles[i] = ps
                sv = ps[:, :RK * H]
                for d in range(D):
                    rhs = bass.AP(tensor=pv.tensor, offset=pv.offset + d * H,
                                  ap=[pv.ap[0], [HD, RK], [1, H]])
                    nc.tensor.matmul(out=sv, lhsT=ident[:], rhs=rhs,
                                     start=(d == 0), stop=(d == D - 1))

            def s3_exp(i):
                (g0, g1, K) = runs[i]
                R = g1 - g0
                RK = R * K
                sv = ps_tiles[i][:, :RK * H]
                w_t = wp.tile([P, RKMAX * 68], BF16, tag="w")
                w_tiles[i] = w_t
                wv = w_t[:, 0:1]
                exo = bass.AP(tensor=wv.tensor, offset=wv.offset + HD,
                              ap=[wv.ap[0], [68 * K, R], [68, K], [1, H]])
                nc.scalar.activation(out=exo, in_=sv, func=ACTF.Exp,
                                     scale=0.25)

            def s4_w(i):
                (g0, g1, K) = runs[i]
                R = g1 - g0
                qv0 = qv_tiles[i][:, 0:1]
                pp = qv0.ap[0]
                wv = w_tiles[i][:, 0:1]
                for r in range(R):
                    wo = bass.AP(tensor=wv.tensor, offset=wv.offset + r * K * 68,
                                 ap=[wv.ap[0], [68, K], [1, HD]])
                    vo = bass.AP(tensor=qv0.tensor,
                                 offset=qv0.offset + r * K * 2 * HD + HD,
                                 ap=[pp, [2 * HD, K], [1, HD]])
                    eb = bass.AP(tensor=wv.tensor,
                                 offset=wv.offset + r * K * 68 + HD,
                                 ap=[wv.ap[0], [68, K], [0, D], [1, H]])
                    nc.vector.tensor_tensor(out=wo, in0=vo, in1=eb,
                                            op=ALU.mult)

            def s5_agg(i):
                (g0, g1, K) = runs[i]
                R = g1 - g0
                wv = w_tiles[i][:, 0:1]
                pa_t = psA.tile([P, RMAX * 68], F32, tag="pa")
                pa_tiles[i] = pa_t
                av = pa_t[:, :R * 68]
                for k in range(K):
                    rhs = bass.AP(tensor=wv.tensor, offset=wv.offset + k * 68,
                                  ap=[wv.ap[0], [68 * K, R], [1, 68]])
                    nc.tensor.matmul(out=av, lhsT=ident[:], rhs=rhs,
                                     start=(k == 0), stop=(k == K - 1))

            def s6_copy(i):
                (g0, g1, K) = runs[i]
                R = g1 - g0
                av = pa_tiles[i][:, :R * 68]
                nc.scalar.activation(
                    out=bass.AP(tensor=agg_sb[:].tensor,
                                offset=agg_sb[:].offset + g0 * HD,
                                ap=[agg_sb[:].ap[0], [HD, R], [1, HD]]),
                    in_=bass.AP(tensor=av.tensor, offset=av.offset,
                                ap=[av.ap[0], [68, R], [1, HD]]),
                    func=ACTF.Copy)
                nc.scalar.activation(
                    out=bass.AP(tensor=den_sb[:].tensor,
                                offset=den_sb[:].offset + g0 * H,
                                ap=[den_sb[:].ap[0], [H, R], [1, H]]),
                    in_=bass.AP(tensor=av.tensor, offset=av.offset + HD,
                                ap=[av.ap[0], [68, R], [1, H]]),
                    func=ACTF.Copy)

            # -------------------------------------------- node-phase chunk
            def node_chunk(ci):
                (_, _, ga, gb) = chunks[ci]
                G = gb - ga
                F = G * HD
                dv = den_sb[:, ga * H:gb * H]
                nc.vector.tensor_tensor(out=dv, in0=dv,
                                        in1=corr_sb[:, ga * H:gb * H],
                                        op=ALU.add)
                nc.vector.reciprocal(out=dv, in_=dv)
                dinv = nodep.tile([P, RMAX * 2 * H], BF16, tag="dinv")
                nc.vector.tensor_scalar(out=dinv[:, :G * H], in0=dv,
                                        scalar1=1.0, scalar2=None,
                                        op0=ALU.mult)
                rst = nodep.tile([P, RMAX * 2 * HD], BF16, tag="rst")
                rv = rst[:, :F]
                r3 = bass.AP(tensor=rv.tensor, offset=rv.offset,
                             ap=[rv.ap[0], [HD, G], [1, HD]])
                a0 = agg_sb[:, 0:1]
                a3 = bass.AP(tensor=a0.tensor, offset=a0.offset + ga * HD,
                             ap=[a0.ap[0], [HD, G], [1, HD]])
                dq = dinv[:, 0:1]
                dinb = bass.AP(tensor=dq.tensor, offset=dq.offset,
                               ap=[dq.ap[0], [H, G], [0, D], [1, H]])
                nc.vector.tensor_tensor(out=r3, in0=a3, in1=dinb, op=ALU.mult)

                skipb = bass.AP(tensor=ksv.tensor,
                                offset=ksv.offset + ga * 2 * HD + HD,
                                ap=[ksv.ap[0], [2 * HD, G], [1, HD]])
                wg2b = bass.AP(tensor=pb.tensor, offset=pb.offset,
                               ap=[pb.ap[0], [0, G], [1, HD]])
                gammab = bass.AP(tensor=pb.tensor, offset=pb.offset + HD,
                                 ap=[pb.ap[0], [0, G], [1, HD]])
                betab = bass.AP(tensor=pb.tensor, offset=pb.offset + 2 * HD,
                                ap=[pb.ap[0], [0, G], [1, HD]])

                z = nodep.tile([P, RMAX * 2 * HD], BF16, tag="z")
                zv = z[:, :F]
                z3 = bass.AP(tensor=zv.tensor, offset=zv.offset,
                             ap=[zv.ap[0], [HD, G], [1, HD]])
                sc = nodep.tile([P, 8 * RMAX * 2], F32, tag="sc")
                r2 = sc[:, 0:G]
                gl = sc[:, G:2 * G]
                vs = sc[:, 2 * G:3 * G]
                xh = sc[:, 3 * G:4 * G]
                t1 = sc[:, 4 * G:5 * G]
                # gate logit: r2 = sum(rst*wg2'), gl = tanh((r1+r2)/2 + bg/2)
                nc.vector.tensor_tensor(out=z3, in0=r3, in1=wg2b, op=ALU.mult)
                nc.vector.tensor_reduce(out=r2, in_=z3, axis=AX.X, op=ALU.add)
                nc.vector.tensor_tensor(out=gl, in0=r2, in1=r1_sb[:, ga:gb],
                                        op=ALU.add)
                nc.scalar.activation(out=gl, in_=gl, func=ACTF.Tanh,
                                     scale=0.5, bias=bg2)
                gate = nodep.tile([P, RMAX * 2], BF16, tag="gate")
                nc.vector.tensor_scalar(out=gate[:, :G], in0=gl, scalar1=0.5,
                                        scalar2=0.5, op0=ALU.mult, op1=ALU.add)
                # rst += gate * (skip - rst)
                dif = nodep.tile([P, RMAX * 2 * HD], BF16, tag="dif")
                dv3 = bass.AP(tensor=dif[:].tensor, offset=dif[:].offset,
                              ap=[dif[:].ap[0], [HD, G], [1, HD]])
                nc.vector.tensor_tensor(out=dv3, in0=skipb, in1=r3,
                                        op=ALU.subtract)
                gq = gate[:, 0:1]
                gb_ = bass.AP(tensor=gq.tensor, offset=gq.offset,
                              ap=[gq.ap[0], [1, G], [0, HD]])
                nc.gpsimd.tensor_tensor(out=dv3, in0=dv3, in1=gb_,
                                        op=ALU.mult)
                nc.vector.tensor_tensor(out=rv, in0=rv, in1=dif[:, :F],
                                        op=ALU.add)
                # LayerNorm: mean
                mu = sc[:, 5 * G:6 * G]
                nc.vector.tensor_reduce(out=mu, in_=r3, axis=AX.X, op=ALU.add)
                mub = bass.AP(tensor=sc[:].tensor, offset=sc[:].offset + 5 * G,
                              ap=[sc[:].ap[0], [1, G], [0, HD]])
                nc.vector.scalar_tensor_tensor(out=rv, in0=mub,
                                               scalar=-1.0 / HD, in1=rv,
                                               op0=ALU.mult, op1=ALU.add)
                # variance -> x = var/64 + eps
                nc.scalar.activation(out=zv, in_=rv, func=ACTF.Square)
                nc.vector.tensor_reduce(out=vs, in_=z3, axis=AX.X, op=ALU.add)
                nc.vector.tensor_scalar(out=vs, in0=vs, scalar1=1.0 / HD,
                                        scalar2=1e-5, op0=ALU.mult,
                                        op1=ALU.add)
                # quake rsqrt: y0 = bits(MAGIC - (i >> 1)); 2 Newton steps
                nc.vector.tensor_scalar(out=xh, in0=vs, scalar1=0.5,
                                        scalar2=None, op0=ALU.mult)
                vi = vs.bitcast(I32)
                nc.vector.tensor_scalar(out=vi, in0=vi, scalar1=1,
                                        scalar2=None,
                                        op0=ALU.logical_shift_right)
                nc.vector.tensor_scalar(out=vi, in0=vi, scalar1=-1,
                                        scalar2=MAGIC, op0=ALU.mult,
                                        op1=ALU.add)
                for _ in range(2):
                    nc.vector.tensor_tensor(out=t1, in0=vs, in1=vs,
                                            op=ALU.mult)
                    nc.vector.tensor_tensor(out=t1, in0=t1, in1=xh,
                                            op=ALU.mult)
                    nc.vector.tensor_scalar(out=t1, in0=t1, scalar1=-1.0,
                                            scalar2=1.5, op0=ALU.mult,
                                            op1=ALU.add)
                    nc.vector.tensor_tensor(out=vs, in0=vs, in1=t1,
                                            op=ALU.mult)
                isd = nodep.tile([P, RMAX * 2], BF16, tag="isd")
                nc.vector.tensor_scalar(out=isd[:, :G], in0=vs, scalar1=1.0,
                                        scalar2=None, op0=ALU.mult)
                iq = isd[:, 0:1]
                isb = bass.AP(tensor=iq.tensor, offset=iq.offset,
                              ap=[iq.ap[0], [1, G], [0, HD]])
                nc.gpsimd.tensor_tensor(out=rv, in0=rv, in1=isb, op=ALU.mult)
                nc.vector.tensor_tensor(out=r3, in0=r3, in1=gammab,
                                        op=ALU.mult)
                nc.vector.tensor_tensor(out=r3, in0=r3, in1=betab, op=ALU.add)
                # PReLU: max(x,0) + a*min(x,0)
                nc.vector.tensor_scalar(out=zv, in0=rv, scalar1=0.0,
                                        scalar2=None, op0=ALU.max)
                nc.vector.tensor_scalar(out=rv, in0=rv, scalar1=0.0,
                                        scalar2=None, op0=ALU.min)
                nc.vector.scalar_tensor_tensor(out=rv, in0=rv, scalar=pa,
                                               in1=zv, op0=ALU.mult,
                                               op1=ALU.add)
                nc.sync.dma_start(out=out_d[:, ga * HD:gb * HD], in_=rv)

            # ------------------------------------------------ emission loop
            chunk_after = {b: ci for ci, (a, b, _, _) in enumerate(chunks)}
            for i in range(nruns):
                s0_dma(i)
                s1_prod(i)
                s2_score(i)
                s3_exp(i)
                if i > 0:
                    s4_w(i - 1)
                    s5_agg(i - 1)
                    s6_copy(i - 1)
                if i in chunk_after:
                    node_chunk(chunk_after[i])
            s4_w(nruns - 1)
            s5_agg(nruns - 1)
            s6_copy(nruns - 1)
            if nruns in chunk_after:
                node_chunk(chunk_after[nruns])

    nc.compile()
    return nc


# ------------------------------------------------------------------- driver

_CACHE = {}


def _get_nc(plan, ncores):
    key = (tuple(plan["Kpad"].tolist()), plan["grid"], ncores)
    if key not in _CACHE:
        _CACHE[key] = _build_nc(plan, ncores)
    return _CACHE[key]


def _make_inmaps(plan, params, ncores):
    (Wk, bk, Wskip, bskip, Wgate, bgate, ln_gamma, ln_beta, prelu_a) = params
    cm = _dh_perm()
    wg = np.asarray(Wgate, np.float32).reshape(3 * HD)
    wg1 = wg[0:64] + wg[128:192]          # acts on skip
    wg2 = wg[64:128] - wg[128:192]        # acts on rst

    wk = np.concatenate([np.asarray(Wk, np.float32),
                         np.asarray(bk, np.float32).reshape(1, HD)])[:, cm]
    wsk_f = np.concatenate([np.asarray(Wskip, np.float32),
                            np.asarray(bskip, np.float32).reshape(1, HD)])
    wsk = wsk_f[:, cm]
    wkws = np.zeros((IN_F + 1, 132), np.float32)
    wkws[:, 0:HD] = wk
    wkws[:, HD:2 * HD] = wsk
    wkws[:, 128] = wsk_f @ wg1            # r1 column (skip-side gate logit)
    wkws = wkws.astype(BF)

    parb = np.zeros((1, 3 * HD), np.float32)
    parb[0, 0:HD] = wg2[cm]
    parb[0, HD:2 * HD] = np.asarray(ln_gamma, np.float32)[cm]
    parb[0, 2 * HD:3 * HD] = np.asarray(ln_beta, np.float32)[cm]
    parb = parb.astype(BF)
    parf = np.array([[np.float32(np.asarray(bgate).reshape(-1)[0]) * 0.5,
                      np.float32(np.asarray(prelu_a).reshape(-1)[0]),
                      0.0]], np.float32)

    in_maps = []
    for c in range(ncores):
        pc = plan["per_core"][c]
        m = dict(featT=plan["featTs"][c], tab=pc["tab"], corr=pc["corr"],
                 ident=plan["ident"], wkws=wkws, parb=parb, parf=parf)
        in_maps.append(m)
    return in_maps


def run(q_src, v_src, feat, src, dst, Wk, bk, Wskip, bskip, Wgate, bgate,
        ln_gamma, ln_beta, prelu_a, ncores=NCORES, trace=False):
    plan = _plan(q_src, v_src, feat, src, dst, ncores)
    nc = _get_nc(plan, ncores)
    in_maps = _make_inmaps(
        plan, (Wk, bk, Wskip, bskip, Wgate, bgate, ln_gamma, ln_beta, prelu_a),
        ncores)
    res = run_bass_kernel_spmd(nc, in_maps, core_ids=list(range(ncores)),
                               trace=trace)
    n, npc, ngrp = plan["n"], plan["npc"], plan["ngrp"]
    out = np.empty((n, HD), np.float32)
    for c in range(ncores):
        r = np.asarray(res.results[c]["out"]).astype(np.float32)
        # [P, ngrp, D, H] -> [ngrp, P, H, D] -> [grid, HD]
        arr = r.reshape(P, ngrp, D, H).transpose(1, 0, 3, 2).reshape(-1, HD)
        out[c * npc + plan["cores"][c]["perm"]] = \
            arr[plan["ndum"]:plan["ndum"] + npc]
    return out, res, plan, in_maps, nc


def kernel(**inputs):
    out, _, _, _, _ = run(**inputs)
    return out
